# revision 30
# baseline (speedup 1.0000x reference)
"""Trainium2 Bass kernel for nn_EstraNet_1443109012284.

Mathematical reduction: the reference's FAVOR+/trig branch (phi_q, aux_q/k,
fr_q/k, aux_A, A) does not feed the output.  The output is exactly

    out[b,n,d] = sum_{h,c} W_o[h,c,d] * norma[h] * sum_{d'} W_v[d',h,c] * x[b,n,d']
               = (x @ M)[b,n,d],   M[d',d] = sum_{h,c} W_v[d',h,c] norma[h] W_o[h,c,d]

with norma[h] = || sum_d s_p[h] W_p[d,h,:] beta_p[d] ||_2.

M is a tiny [512,512] matrix folded on the host; the device does the single
big GEMM  y[32768,512] = x[32768,512] @ M[512,512]  data-parallel over rows:
each of the 8 cores handles 4096 rows.

Device design (per core): compute yT[d, n] = sum_k M[k, d] * xT[k, n]
- lhsT (stationary) = M chunk [128k x 128d]; rhs (moving) = xT slices
  [128k x 512n], fed pre-transposed from the host (no on-device transpose).
- All input DMAs on the single sync HWDGE ring, ordered by need:
  m(d 0:256), x(k0..k3, h0) quarters, m(d 256:512), then one 1MB DMA per
  remaining h.  A single ring delivers strictly in order at line rate, so
  the first matmul can fire after just 0.5MB has landed.
- First compute block covers (h0, d0+d1) with k OUTERMOST over 4 PSUM
  banks: each k-row is 4 matmuls (0.87us) against a 0.66us quarter arrival
  cadence -> the PE streams with no stalls from the first matmul on.
  Remaining (h,d) phases are k-major, j-inner as usual (2 banks each).
- PSUM->SBUF copies all on ONE engine (ACT) while the PE is running:
  PE drain + a single reader share PSUM fine; two concurrent readers
  throttle the PE ~2.3x.  Only in the final phase (PE idle) copies split
  ACT||DVE and the last output DMAs split across both HWDGE rings to
  shorten the drain.
- PE warmed up with dummy matmuls (dep: a memset tile only) during the
  input-DMA window so the HAM clock ramp doesn't tax real work.
- fp16 path (default): x, M, y all fp16, M pre-scaled by an exact power of
  two so M / y avoid the fp16 subnormal range; host multiplies the scale
  back out.  fp16 keeps 10 mantissa bits (vs bf16's 7) and halves output
  DMA vs fp32 -> kernel is PE-bound at ~216ns per [128x128]x[128x512] MM.
"""

import os as _os
import sys

sys.path.insert(0, "/opt/trn_rl_repo")

import numpy as np

import concourse.bass as bass
import concourse.tile as tile
from concourse import bacc, mybir
from concourse.bass_utils import run_bass_kernel_spmd

N_CORES = 8
ROWS = 32768           # B*N = 8*4096
RPC = ROWS // N_CORES  # rows per core = 4096
D = 512
KC = 4                 # contraction chunks of 128
DT = D // 128          # output row-blocks = 4
HB = 4                 # n-quarters per stripe
HW = RPC // HB         # 1024 columns per quarter
JH = HW // 512         # moving chunks of 512 per phase = 2

COMPUTE_DTYPE = _os.environ.get("KERNEL_DTYPE", "fp16")
N_WARM = int(_os.environ.get("KERNEL_NWARM", "10"))

_DT = {
    "fp32": mybir.dt.float32,
    "f32r": mybir.dt.float32r,
    "bf16": mybir.dt.bfloat16,
    "fp16": mybir.dt.float16,
}


def _np_dtype(token):
    if token == "bf16":
        import ml_dtypes

        return ml_dtypes.bfloat16
    if token == "fp16":
        return np.float16
    return np.float32


def _build(token):
    dt_in = _DT[token]
    dt_out = mybir.dt.float16 if token == "fp16" else mybir.dt.float32
    nc = bacc.Bacc("TRN2", target_bir_lowering=False)
    # x pre-transposed, [h, partition, k-chunk, column]: h0 loads as two
    # 512KB k-pair pieces, h1..h3 as two 512KB j-half (column-half) pieces
    xt = nc.dram_tensor("xt", [HB, 128, KC, HW], dt_in, kind="ExternalInput")
    # m split in two d-halves of 256 output columns each; [p, q, k, c]
    mm = nc.dram_tensor("mm", [128, 2, KC, 256], dt_in, kind="ExternalInput")
    yt = nc.dram_tensor("yt", [D, RPC], dt_out, kind="ExternalOutput")

    def wsl(m_sb, k, d):
        # stationary weights for (k-chunk, d-block): [128, 128]
        q, r = divmod(d, 2)
        return m_sb[:, q, k, r * 128 : (r + 1) * 128]

    with tile.TileContext(nc) as tc:
        with (
            tc.tile_pool(name="xp", bufs=1) as xp,
            tc.tile_pool(name="mp", bufs=1) as mp,
            tc.tile_pool(name="op", bufs=6) as op,
            tc.tile_pool(name="pp", bufs=8, space="PSUM") as pp,
        ):
            # PE warmup: matmuls that depend only on a memset tile start
            # right after engine code load and burn the HAM cold-clock ramp
            # while the input DMAs are still in flight.  Two alternating PSUM
            # banks so consecutive warmups pipeline at the cold issue rate
            # (~427ns) instead of serializing on a same-bank WAW (~630ns);
            # sized to keep the PE gaplessly busy through the HAM un-throttle
            # point (~3.4us after the first warmup).
            wz = mp.tile([128, 512], mybir.dt.bfloat16, name="wz")
            nc.gpsimd.memset(wz[:], 1.0)
            warms = [
                pp.tile([128, 512], mybir.dt.float32, tag="ps", name=f"warm{i}")
                for i in range(2)
            ]
            for w in range(N_WARM):
                nc.tensor.matmul(
                    warms[w % 2][:], wz[:, 0:128], wz[:], start=True, stop=True
                )

            # --- input DMAs.  Constraints learned from traces:
            # (1) a ring's completion-sem descriptor is ordered IN the ring,
            #     so each piece's HBM-write-receipt (~1.2us) stalls the next
            #     piece: per-ring cadence ~2us for 256KB pieces.  Fewer,
            #     bigger pieces amortize the stall;
            # (2) only ~8 dma_starts issue unguarded (8 HW-DMA sem lanes,
            #     recycled with completion waits);
            # (3) need-order within a ring is delivery order (FIFO), and the
            #     scalar (ACT) ring runs ~1us behind the sync (SP) ring.
            # Critical set = m0 + h0: m0/K01 as ring-parallel first pieces,
            # K23 right behind -> all of h0 usable by ~12.7us.
            m_sb = mp.tile([128, 2, KC, 256], dt_in, name="m_sb")
            x_sb = [
                xp.tile([128, KC, HW], dt_in, tag=f"xh{h}", name=f"xh{h}")
                for h in range(HB)
            ]

            def kfeed(eng, h, k0, nk):
                eng.dma_start(out=x_sb[h][:, k0 : k0 + nk], in_=xt[h][:, k0 : k0 + nk])

            def jfeed(eng, h, half):
                c0, c1 = half * 512, half * 512 + 512
                eng.dma_start(out=x_sb[h][:, :, c0:c1], in_=xt[h][:, :, c0:c1])

            nc.sync.dma_start(out=m_sb[:, 0], in_=mm[:, 0])        # m0: d 0:256
            kfeed(nc.sync, 0, 0, 1)                                # h0 k0
            kfeed(nc.sync, 0, 1, 1)                                # h0 k1
            kfeed(nc.scalar, 0, 2, 1)                              # h0 k2
            kfeed(nc.scalar, 0, 3, 1)                              # h0 k3
            nc.scalar.dma_start(out=m_sb[:, 1], in_=mm[:, 1])      # m1: d 256:512
            jfeed(nc.scalar, 1, 0)                                 # h1 j-half 0
            jfeed(nc.sync, 1, 1)                                   # h1 j-half 1
            jfeed(nc.scalar, 2, 0)                                 # h2 j-half 0
            jfeed(nc.sync, 2, 1)                                   # h2 j-half 1
            jfeed(nc.scalar, 3, 0)                                 # h3 j-half 0
            jfeed(nc.sync, 3, 1)                                   # h3 j-half 1

            def xs(h, k, j):
                return x_sb[h][:, k, j * 512 : (j + 1) * 512]

            # --- block A: (h0, d0+d1), k outermost over 4 PSUM banks ---
            # 4 MMs per k-row > one quarter's arrival time -> no PE stalls
            # while h0's k-slices stream in.
            psA = {
                (d, j): pp.tile([128, 512], mybir.dt.float32, tag="ps", name=f"psA_{d}_{j}")
                for d in range(2)
                for j in range(JH)
            }
            # k-row order matched to piece arrival: k2 (scalar-1) lands first,
            # then k0 (sync-2), k3 (scalar-2), k1 (sync-3)
            A_ORDER = (2, 0, 3, 1)
            for ki, k in enumerate(A_ORDER):
                for d in range(2):
                    for j in range(JH):
                        nc.tensor.matmul(
                            psA[(d, j)][:],
                            wsl(m_sb, k, d),
                            xs(0, k, j),
                            start=(ki == 0),
                            stop=(ki == KC - 1),
                        )
            for d in range(2):
                ot = op.tile([128, HW], dt_out, name=f"otA{d}", tag="ot")
                for j in range(JH):
                    nc.scalar.copy(ot[:, j * 512 : (j + 1) * 512], psA[(d, j)][:])
                oeng = nc.sync if d == 0 else nc.scalar
                oeng.dma_start(out=yt[d * 128 : (d + 1) * 128, 0:HW], in_=ot[:])

            # --- remaining phases: k-major, j-inner, 2 banks each ---
            rest = [(0, 2), (0, 3)] + [(h, d) for h in range(1, HB) for d in range(DT)]
            for ph, (h, d) in enumerate(rest):
                d0 = d * 128
                last = ph == len(rest) - 1
                ot = op.tile([128, HW], dt_out, name=f"ot{ph}", tag="ot")
                if last:
                    # final phase, j-major: j0's copy+store overlap j1's MMs;
                    # j1 drains as ONE DVE copy (starts the instant the last
                    # MM retires — the idle DVE has no FIFO backlog) + ONE
                    # store on the fast sync ring, whose late queue was kept
                    # clear of other output pieces.
                    pss = [
                        pp.tile([128, 512], mybir.dt.float32, tag="ps", name=f"psl{j}")
                        for j in range(JH)
                    ]
                    for k in range(KC):
                        nc.tensor.matmul(
                            pss[0][:],
                            wsl(m_sb, k, d),
                            xs(h, k, 0),
                            start=(k == 0),
                            stop=(k == KC - 1),
                        )
                    # j0's store on the SAME engine as its copy: the engine's
                    # FIFO guarantees it issues right after the copy, instead
                    # of being re-ordered behind j1's store by the scheduler
                    nc.scalar.copy(ot[:, 0:512], pss[0][:])
                    nc.scalar.dma_start(
                        out=yt[d0 : d0 + 128, h * HW : h * HW + 512],
                        in_=ot[:, 0:512],
                    )
                    for k in range(KC):
                        nc.tensor.matmul(
                            pss[1][:],
                            wsl(m_sb, k, d),
                            xs(h, k, 1),
                            start=(k == 0),
                            stop=(k == KC - 1),
                        )
                    nc.vector.tensor_copy(ot[:, 512:1024], pss[1][:])
                    nc.sync.dma_start(
                        out=yt[d0 : d0 + 128, h * HW + 512 : (h + 1) * HW],
                        in_=ot[:, 512:1024],
                    )
                else:
                    pss = [
                        pp.tile([128, 512], mybir.dt.float32, tag="ps", name=f"ps_{h}_{d}_{j}")
                        for j in range(JH)
                    ]
                    # j-major: a phase's j0 block only needs the h-stripe's
                    # first column-half, and its copy overlaps the j1 block
                    for j in range(JH):
                        for k in range(KC):
                            nc.tensor.matmul(
                                pss[j][:],
                                wsl(m_sb, k, d),
                                xs(h, k, j),
                                start=(k == 0),
                                stop=(k == KC - 1),
                            )
                        nc.scalar.copy(ot[:, j * 512 : (j + 1) * 512], pss[j][:])
                    # alternate rings, but keep the last pre-final outputs off
                    # the sync ring so the final pieces aren't queued behind
                    # them
                    oeng = nc.scalar if ph >= 11 else (nc.sync if ph % 2 == 0 else nc.scalar)
                    oeng.dma_start(
                        out=yt[d0 : d0 + 128, h * HW : (h + 1) * HW], in_=ot[:]
                    )
    nc.compile()
    return nc


def _fold_m(W_v, s_p, W_p, beta_p, W_o):
    """Host-side constant folding of the tiny parameter tensors into M."""
    W_v = np.asarray(W_v, dtype=np.float64)
    s_p = np.asarray(s_p, dtype=np.float64)
    W_p = np.asarray(W_p, dtype=np.float64)
    beta_p = np.asarray(beta_p, dtype=np.float64)
    W_o = np.asarray(W_o, dtype=np.float64)
    phi = np.einsum("h,dhc,d->hc", s_p, W_p, beta_p)
    norma = np.linalg.norm(phi, axis=1)  # [h]
    M = np.einsum("dhc,h,hce->de", W_v, norma, W_o)  # [512, 512]
    return M.astype(np.float32)


_prog_cache = {}
_last_in_maps = None  # kept for test.py profiling reuse
_last_result = None


def _run(in_maps, token, **kwargs):
    if token not in _prog_cache:
        _prog_cache[token] = _build(token)
    return run_bass_kernel_spmd(_prog_cache[token], in_maps, list(range(N_CORES)), **kwargs)


def kernel(x, W_v, s_p, c_p, W_p, W_A, W_o, beta_p, beta_i_p, **_unused):
    global _last_in_maps, _last_result
    token = COMPUTE_DTYPE
    np_dt = _np_dtype(token)

    x = np.asarray(x, dtype=np.float32)
    M = _fold_m(W_v, s_p, W_p, beta_p, W_o)

    # fp16 path: scale M by an exact power of two so M entries and y values
    # sit in fp16 normal range; undo on the host after the run
    out_unscale = 1.0
    if token == "fp16":
        amax = float(np.abs(M).max())
        if amax > 0:
            e = int(np.floor(-np.log2(amax)))
            M = M * np.float32(2.0**e)
            out_unscale = 2.0**-e

    B, N, Dd = x.shape
    assert B * N == ROWS and Dd == D, (x.shape,)

    # [128, KC, D] with partition dim first, then split D into two halves:
    # mm2[p, q, k, c] = M[k*128+p, q*256+c]
    mmc = np.ascontiguousarray(M.reshape(KC, 128, D).transpose(1, 0, 2)).astype(np_dt)
    mm2 = np.ascontiguousarray(mmc.reshape(128, KC, 2, 256).transpose(0, 2, 1, 3))
    xf = x.reshape(ROWS, D)

    in_maps = []
    for c in range(N_CORES):
        sh = xf[c * RPC : (c + 1) * RPC]               # [4096, 512]
        xT = sh.T.astype(np_dt)                        # [512, 4096]
        # [KC, 128, HB, HW] -> [HB, 128, KC, HW]
        xs = np.ascontiguousarray(
            xT.reshape(KC, 128, HB, HW).transpose(2, 1, 0, 3)
        )
        in_maps.append({"xt": xs, "mm": mm2})

    _last_in_maps = in_maps
    res = _run(in_maps, token)
    _last_result = res
    out = np.empty((ROWS, D), dtype=np.float32)
    for c in range(N_CORES):
        yc = res.results[c]["yt"].astype(np.float32)
        if out_unscale != 1.0:
            yc *= np.float32(out_unscale)
        out[c * RPC : (c + 1) * RPC] = yc.T
    return out.reshape(B, N, D)


if __name__ == "__main__":
    # smoke test with random data
    rng = np.random.default_rng(0)
    x = rng.standard_normal((8, 4096, 512)).astype(np.float32)
    W_v = rng.standard_normal((512, 8, 64)).astype(np.float32) * 0.01
    s_p = np.ones((8,), np.float32)
    c_p = np.ones((8,), np.float32)
    W_p = rng.standard_normal((512, 8, 64)).astype(np.float32) * 0.01
    W_A = rng.standard_normal((256, 64)).astype(np.float32)
    W_o = rng.standard_normal((8, 64, 512)).astype(np.float32) * 0.01
    beta_p = rng.standard_normal((512,)).astype(np.float32) * 1e-5
    beta_i_p = rng.standard_normal((4096, 512)).astype(np.float32) * 1e-5
    out = kernel(x, W_v=W_v, s_p=s_p, c_p=c_p, W_p=W_p, W_A=W_A, W_o=W_o,
                 beta_p=beta_p, beta_i_p=beta_i_p)
    M = _fold_m(W_v, s_p, W_p, beta_p, W_o)
    exp = (x.reshape(-1, 512).astype(np.float64) @ M.astype(np.float64)).reshape(8, 4096, 512)
    err = np.abs(out - exp).max() / (np.abs(exp).max() + 1e-30)
    print("smoke rel err:", err)


# revision 31
# speedup vs baseline: 1.1518x; 1.1518x over previous
"""Trainium2 Bass kernel for nn_EstraNet_1443109012284.

Mathematical reduction: the reference's FAVOR+/trig branch (phi_q, aux_q/k,
fr_q/k, aux_A, A) does not feed the output.  The output is exactly

    out[b,n,d] = sum_{h,c} W_o[h,c,d] * norma[h] * sum_{d'} W_v[d',h,c] * x[b,n,d']
               = (x @ M)[b,n,d],   M[d',d] = sum_{h,c} W_v[d',h,c] norma[h] W_o[h,c,d]

with norma[h] = || sum_d s_p[h] W_p[d,h,:] beta_p[d] ||_2.

M is a tiny [512,512] matrix folded on the host; the device does the single
big GEMM  y[32768,512] = x[32768,512] @ M[512,512]  data-parallel over rows:
each of the 8 cores handles 4096 rows.

Device design (per core): compute yT[d, n] = sum_k M[k, d] * xT[k, n]
- lhsT (stationary) = M chunk [128k x 128d]; rhs (moving) = xT slices
  [128k x 512n], fed pre-transposed from the host (no on-device transpose).
- All input DMAs on the single sync HWDGE ring, ordered by need:
  m(d 0:256), x(k0..k3, h0) quarters, m(d 256:512), then one 1MB DMA per
  remaining h.  A single ring delivers strictly in order at line rate, so
  the first matmul can fire after just 0.5MB has landed.
- First compute block covers (h0, d0+d1) with k OUTERMOST over 4 PSUM
  banks: each k-row is 4 matmuls (0.87us) against a 0.66us quarter arrival
  cadence -> the PE streams with no stalls from the first matmul on.
  Remaining (h,d) phases are k-major, j-inner as usual (2 banks each).
- PSUM->SBUF copies all on ONE engine (ACT) while the PE is running:
  PE drain + a single reader share PSUM fine; two concurrent readers
  throttle the PE ~2.3x.  Only in the final phase (PE idle) copies split
  ACT||DVE and the last output DMAs split across both HWDGE rings to
  shorten the drain.
- PE warmed up with dummy matmuls (dep: a memset tile only) during the
  input-DMA window so the HAM clock ramp doesn't tax real work.
- fp16 path (default): x, M, y all fp16, M pre-scaled by an exact power of
  two so M / y avoid the fp16 subnormal range; host multiplies the scale
  back out.  fp16 keeps 10 mantissa bits (vs bf16's 7) and halves output
  DMA vs fp32 -> kernel is PE-bound at ~216ns per [128x128]x[128x512] MM.
"""

import os as _os
import sys

sys.path.insert(0, "/opt/trn_rl_repo")

import numpy as np

import concourse.bass as bass
import concourse.tile as tile
from concourse import bacc, mybir
from concourse.bass_utils import run_bass_kernel_spmd

N_CORES = 8
ROWS = 32768           # B*N = 8*4096
RPC = ROWS // N_CORES  # rows per core = 4096
D = 512
KC = 4                 # contraction chunks of 128
DT = D // 128          # output row-blocks = 4
HB = 4                 # n-quarters per stripe
HW = RPC // HB         # 1024 columns per quarter
JH = HW // 512         # moving chunks of 512 per phase = 2

COMPUTE_DTYPE = _os.environ.get("KERNEL_DTYPE", "fp16")
N_WARM = int(_os.environ.get("KERNEL_NWARM", "10"))

_DT = {
    "fp32": mybir.dt.float32,
    "f32r": mybir.dt.float32r,
    "bf16": mybir.dt.bfloat16,
    "fp16": mybir.dt.float16,
}


def _np_dtype(token):
    if token == "bf16":
        import ml_dtypes

        return ml_dtypes.bfloat16
    if token == "fp16":
        return np.float16
    return np.float32


def _build(token):
    dt_in = _DT[token]
    dt_out = mybir.dt.float16 if token == "fp16" else mybir.dt.float32
    nc = bacc.Bacc("TRN2", target_bir_lowering=False)
    # x pre-transposed, [h, partition, k-chunk, column]: h0 loads as two
    # 512KB k-pair pieces, h1..h3 as two 512KB j-half (column-half) pieces
    xt = nc.dram_tensor("xt", [HB, 128, KC, HW], dt_in, kind="ExternalInput")
    # m split in two d-halves of 256 output columns each; [p, q, k, c]
    mm = nc.dram_tensor("mm", [128, 2, KC, 256], dt_in, kind="ExternalInput")
    yt = nc.dram_tensor("yt", [D, RPC], dt_out, kind="ExternalOutput")

    def wsl(m_sb, k, d):
        # stationary weights for (k-chunk, d-block): [128, 128]
        q, r = divmod(d, 2)
        return m_sb[:, q, k, r * 128 : (r + 1) * 128]

    with tile.TileContext(nc) as tc:
        with (
            tc.tile_pool(name="xp", bufs=1) as xp,
            tc.tile_pool(name="mp", bufs=1) as mp,
            tc.tile_pool(name="op", bufs=6) as op,
            tc.tile_pool(name="pp", bufs=8, space="PSUM") as pp,
        ):
            # PE warmup: matmuls that depend only on a memset tile start
            # right after engine code load and burn the HAM cold-clock ramp
            # while the input DMAs are still in flight.  Two alternating PSUM
            # banks so consecutive warmups pipeline at the cold issue rate
            # (~427ns) instead of serializing on a same-bank WAW (~630ns);
            # sized to keep the PE gaplessly busy through the HAM un-throttle
            # point (~3.4us after the first warmup).
            wz = mp.tile([128, 512], mybir.dt.bfloat16, name="wz")
            nc.gpsimd.memset(wz[:], 1.0)
            warms = [
                pp.tile([128, 512], mybir.dt.float32, tag="ps", name=f"warm{i}")
                for i in range(2)
            ]
            for w in range(N_WARM):
                nc.tensor.matmul(
                    warms[w % 2][:], wz[:, 0:128], wz[:], start=True, stop=True
                )

            # --- input DMAs.  Constraints learned from traces:
            # (1) a ring's completion-sem descriptor is ordered IN the ring,
            #     so each piece's HBM-write-receipt (~1.2us) stalls the next
            #     piece: per-ring cadence ~2us for 256KB pieces.  Fewer,
            #     bigger pieces amortize the stall;
            # (2) only ~8 dma_starts issue unguarded (8 HW-DMA sem lanes,
            #     recycled with completion waits);
            # (3) need-order within a ring is delivery order (FIFO), and the
            #     scalar (ACT) ring runs ~1us behind the sync (SP) ring.
            # Critical set = m0 + h0: m0/K01 as ring-parallel first pieces,
            # K23 right behind -> all of h0 usable by ~12.7us.
            m_sb = mp.tile([128, 2, KC, 256], dt_in, name="m_sb")
            x_sb = [
                xp.tile([128, KC, HW], dt_in, tag=f"xh{h}", name=f"xh{h}")
                for h in range(HB)
            ]

            def kfeed(eng, h, k0, nk):
                eng.dma_start(out=x_sb[h][:, k0 : k0 + nk], in_=xt[h][:, k0 : k0 + nk])

            def jfeed(eng, h, half):
                c0, c1 = half * 512, half * 512 + 512
                eng.dma_start(out=x_sb[h][:, :, c0:c1], in_=xt[h][:, :, c0:c1])

            nc.sync.dma_start(out=m_sb[:, 0], in_=mm[:, 0])        # m0: d 0:256
            kfeed(nc.sync, 0, 0, 1)                                # h0 k0
            kfeed(nc.sync, 0, 1, 1)                                # h0 k1
            nc.sync.dma_start(out=m_sb[:, 1], in_=mm[:, 1])        # m1: d 256:512
            kfeed(nc.scalar, 0, 2, 1)                              # h0 k2
            kfeed(nc.scalar, 0, 3, 1)                              # h0 k3
            jfeed(nc.scalar, 1, 0)                                 # h1 j-half 0
            jfeed(nc.sync, 1, 1)                                   # h1 j-half 1
            jfeed(nc.scalar, 2, 0)                                 # h2 j-half 0
            jfeed(nc.sync, 2, 1)                                   # h2 j-half 1
            jfeed(nc.scalar, 3, 0)                                 # h3 j-half 0
            jfeed(nc.sync, 3, 1)                                   # h3 j-half 1

            def xs(h, k, j):
                return x_sb[h][:, k, j * 512 : (j + 1) * 512]

            # --- block A: (h0, d0+d1), k outermost over 4 PSUM banks ---
            # 4 MMs per k-row > one quarter's arrival time -> no PE stalls
            # while h0's k-slices stream in.
            psA = {
                (d, j): pp.tile([128, 512], mybir.dt.float32, tag="ps", name=f"psA_{d}_{j}")
                for d in range(2)
                for j in range(JH)
            }
            # k-row order matched to piece arrival: k2 (scalar-1) lands first,
            # then k0 (sync-2), k3 (scalar-2), k1 (sync-3)
            A_ORDER = (2, 0, 3, 1)
            for ki, k in enumerate(A_ORDER):
                for d in range(2):
                    for j in range(JH):
                        nc.tensor.matmul(
                            psA[(d, j)][:],
                            wsl(m_sb, k, d),
                            xs(0, k, j),
                            start=(ki == 0),
                            stop=(ki == KC - 1),
                        )
            for d in range(2):
                ot = op.tile([128, HW], dt_out, name=f"otA{d}", tag="ot")
                for j in range(JH):
                    nc.scalar.copy(ot[:, j * 512 : (j + 1) * 512], psA[(d, j)][:])
                oeng = nc.sync if d == 0 else nc.scalar
                oeng.dma_start(out=yt[d * 128 : (d + 1) * 128, 0:HW], in_=ot[:])

            # --- remaining phases: k-major, j-inner, 2 banks each ---
            rest = [(0, 2), (0, 3)] + [(h, d) for h in range(1, HB) for d in range(DT)]
            for ph, (h, d) in enumerate(rest):
                d0 = d * 128
                last = ph == len(rest) - 1
                ot = op.tile([128, HW], dt_out, name=f"ot{ph}", tag="ot")
                if last:
                    # final phase, j-major: j0's copy+store overlap j1's MMs;
                    # j1 drains as ONE DVE copy (starts the instant the last
                    # MM retires — the idle DVE has no FIFO backlog) + ONE
                    # store on the fast sync ring, whose late queue was kept
                    # clear of other output pieces.
                    pss = [
                        pp.tile([128, 512], mybir.dt.float32, tag="ps", name=f"psl{j}")
                        for j in range(JH)
                    ]
                    for k in range(KC):
                        nc.tensor.matmul(
                            pss[0][:],
                            wsl(m_sb, k, d),
                            xs(h, k, 0),
                            start=(k == 0),
                            stop=(k == KC - 1),
                        )
                    # j0's store on the SAME engine as its copy: the engine's
                    # FIFO guarantees it issues right after the copy, instead
                    # of being re-ordered behind j1's store by the scheduler
                    nc.scalar.copy(ot[:, 0:512], pss[0][:])
                    nc.scalar.dma_start(
                        out=yt[d0 : d0 + 128, h * HW : h * HW + 512],
                        in_=ot[:, 0:512],
                    )
                    for k in range(KC):
                        nc.tensor.matmul(
                            pss[1][:],
                            wsl(m_sb, k, d),
                            xs(h, k, 1),
                            start=(k == 0),
                            stop=(k == KC - 1),
                        )
                    nc.vector.tensor_copy(ot[:, 512:1024], pss[1][:])
                    nc.sync.dma_start(
                        out=yt[d0 : d0 + 128, h * HW + 512 : (h + 1) * HW],
                        in_=ot[:, 512:1024],
                    )
                else:
                    pss = [
                        pp.tile([128, 512], mybir.dt.float32, tag="ps", name=f"ps_{h}_{d}_{j}")
                        for j in range(JH)
                    ]
                    # j-major: a phase's j0 block only needs the h-stripe's
                    # first column-half, and its copy overlaps the j1 block
                    for j in range(JH):
                        for k in range(KC):
                            nc.tensor.matmul(
                                pss[j][:],
                                wsl(m_sb, k, d),
                                xs(h, k, j),
                                start=(k == 0),
                                stop=(k == KC - 1),
                            )
                        nc.scalar.copy(ot[:, j * 512 : (j + 1) * 512], pss[j][:])
                    # alternate rings, but keep the last pre-final outputs off
                    # the sync ring so the final pieces aren't queued behind
                    # them
                    oeng = nc.scalar if ph >= 11 else (nc.sync if ph % 2 == 0 else nc.scalar)
                    oeng.dma_start(
                        out=yt[d0 : d0 + 128, h * HW : (h + 1) * HW], in_=ot[:]
                    )
    nc.compile()
    return nc


def _fold_m(W_v, s_p, W_p, beta_p, W_o):
    """Host-side constant folding of the tiny parameter tensors into M."""
    W_v = np.asarray(W_v, dtype=np.float64)
    s_p = np.asarray(s_p, dtype=np.float64)
    W_p = np.asarray(W_p, dtype=np.float64)
    beta_p = np.asarray(beta_p, dtype=np.float64)
    W_o = np.asarray(W_o, dtype=np.float64)
    phi = np.einsum("h,dhc,d->hc", s_p, W_p, beta_p)
    norma = np.linalg.norm(phi, axis=1)  # [h]
    M = np.einsum("dhc,h,hce->de", W_v, norma, W_o)  # [512, 512]
    return M.astype(np.float32)


_prog_cache = {}
_last_in_maps = None  # kept for test.py profiling reuse
_last_result = None


def _run(in_maps, token, **kwargs):
    if token not in _prog_cache:
        _prog_cache[token] = _build(token)
    return run_bass_kernel_spmd(_prog_cache[token], in_maps, list(range(N_CORES)), **kwargs)


def kernel(x, W_v, s_p, c_p, W_p, W_A, W_o, beta_p, beta_i_p, **_unused):
    global _last_in_maps, _last_result
    token = COMPUTE_DTYPE
    np_dt = _np_dtype(token)

    x = np.asarray(x, dtype=np.float32)
    M = _fold_m(W_v, s_p, W_p, beta_p, W_o)

    # fp16 path: scale M by an exact power of two so M entries and y values
    # sit in fp16 normal range; undo on the host after the run
    out_unscale = 1.0
    if token == "fp16":
        amax = float(np.abs(M).max())
        if amax > 0:
            e = int(np.floor(-np.log2(amax)))
            M = M * np.float32(2.0**e)
            out_unscale = 2.0**-e

    B, N, Dd = x.shape
    assert B * N == ROWS and Dd == D, (x.shape,)

    # [128, KC, D] with partition dim first, then split D into two halves:
    # mm2[p, q, k, c] = M[k*128+p, q*256+c]
    mmc = np.ascontiguousarray(M.reshape(KC, 128, D).transpose(1, 0, 2)).astype(np_dt)
    mm2 = np.ascontiguousarray(mmc.reshape(128, KC, 2, 256).transpose(0, 2, 1, 3))
    xf = x.reshape(ROWS, D)

    in_maps = []
    for c in range(N_CORES):
        sh = xf[c * RPC : (c + 1) * RPC]               # [4096, 512]
        xT = sh.T.astype(np_dt)                        # [512, 4096]
        # [KC, 128, HB, HW] -> [HB, 128, KC, HW]
        xs = np.ascontiguousarray(
            xT.reshape(KC, 128, HB, HW).transpose(2, 1, 0, 3)
        )
        in_maps.append({"xt": xs, "mm": mm2})

    _last_in_maps = in_maps
    res = _run(in_maps, token)
    _last_result = res
    out = np.empty((ROWS, D), dtype=np.float32)
    for c in range(N_CORES):
        yc = res.results[c]["yt"].astype(np.float32)
        if out_unscale != 1.0:
            yc *= np.float32(out_unscale)
        out[c * RPC : (c + 1) * RPC] = yc.T
    return out.reshape(B, N, D)


if __name__ == "__main__":
    # smoke test with random data
    rng = np.random.default_rng(0)
    x = rng.standard_normal((8, 4096, 512)).astype(np.float32)
    W_v = rng.standard_normal((512, 8, 64)).astype(np.float32) * 0.01
    s_p = np.ones((8,), np.float32)
    c_p = np.ones((8,), np.float32)
    W_p = rng.standard_normal((512, 8, 64)).astype(np.float32) * 0.01
    W_A = rng.standard_normal((256, 64)).astype(np.float32)
    W_o = rng.standard_normal((8, 64, 512)).astype(np.float32) * 0.01
    beta_p = rng.standard_normal((512,)).astype(np.float32) * 1e-5
    beta_i_p = rng.standard_normal((4096, 512)).astype(np.float32) * 1e-5
    out = kernel(x, W_v=W_v, s_p=s_p, c_p=c_p, W_p=W_p, W_A=W_A, W_o=W_o,
                 beta_p=beta_p, beta_i_p=beta_i_p)
    M = _fold_m(W_v, s_p, W_p, beta_p, W_o)
    exp = (x.reshape(-1, 512).astype(np.float64) @ M.astype(np.float64)).reshape(8, 4096, 512)
    err = np.abs(out - exp).max() / (np.abs(exp).max() + 1e-30)
    print("smoke rel err:", err)
